# revision 1
# baseline (speedup 1.0000x reference)
"""Pairwise cosine-similarity kernel for Trainium2 (8 NeuronCores, SPMD).

Computes out = 16 * normalize(x1) @ normalize(x2).T for x1, x2 [8192, 512] f32.

Sharding: x1 rows are split across the 8 cores (1024 rows each); x2 is
replicated. Each core computes its [1024, 8192] slice of the output; the host
concatenates the slices.

Host-side prep is layout/dtype only: inputs are cast to bf16 and x2 is
additionally shipped pre-transposed ([512, 8192]) so the big operand needs no
on-device transposition. All FLOPs (norms, normalization, GEMM, scaling) run
on device:

  1. x1 (bf16, natural): fused Square+row-sum on ScalarE -> sqrt -> clamp ->
     reciprocal -> x1n = x1 * (16/n1) via per-partition tensor_scalar, then
     PE-transpose (bf16 matmul vs. identity) into x1T [D, rows].
  2. x2 norms from the natural-layout bf16 copy (per column-group of 2048
     rows): Square+row-sum, sqrt, clamp, reciprocal -> inv2 [128, 16] compact.
     PE-transpose inv2 to [16, 128], then broadcast across partitions with
     K=1 ones-matmuls -> inv2_bcast [128, 2048] f32, and scale the
     pre-transposed x2T tiles in place (DVE tensor_tensor).
  3. Main GEMM: out_tile[128, 512] += x1T.T @ x2T over 4 K-chunks (bf16,
     f32 PSUM), PSUM->SBUF copies split across DVE/ACT, DMA out.
"""

import sys

for _p in ("/root/.axon_site/_ro/trn_rl_repo", "/opt/trn_rl_repo"):
    if _p not in sys.path:
        sys.path.append(_p)

import ml_dtypes
import numpy as np

import concourse.bass as bass
import concourse.tile as tile
from concourse import bacc, mybir
from concourse.bass_utils import run_bass_kernel_spmd
from concourse.masks import make_identity

F32 = mybir.dt.float32
BF16 = mybir.dt.bfloat16
P = 128
SCALE = 16.0
EPS = 1e-8

N_CORES = 8
N1 = 8192  # x1 rows (total)
N2 = 8192  # x2 rows
D = 512  # feature dim

_PROGRAM_CACHE = {}


def build_program(n1_local=N1 // N_CORES, n2=N2, d=D, cg_width=1024):
    """Build the SPMD program one core runs. Returns the compiled Bacc.

    DRAM inputs: x1 [n1_local, d] bf16 (natural), x2n [n2, d] bf16 (natural,
    norms only), x2t [d, n2] bf16 (pre-transposed, GEMM operand).
    """
    kc = d // P  # K-chunks of the contraction dim
    m_tiles = n1_local // P  # x1 row-tiles per core
    n_cgs = n2 // cg_width  # output column groups
    nch = cg_width // 512  # 512-wide chunks per column group
    cg_rt = cg_width // P  # x2 row-tiles per column group

    nc = bacc.Bacc("TRN2", target_bir_lowering=False, debug=False,
                   num_devices=N_CORES)
    x1 = nc.dram_tensor("x1", [n1_local, d], BF16, kind="ExternalInput")
    x2n = nc.dram_tensor("x2n", [n2, d], BF16, kind="ExternalInput")
    x2t = nc.dram_tensor("x2t", [d, n2], BF16, kind="ExternalInput")
    out = nc.dram_tensor("out", [n1_local, n2], F32, kind="ExternalOutput")

    with tile.TileContext(nc) as tc:
        with (
            tc.tile_pool(name="const", bufs=1) as const,
            tc.tile_pool(name="ld", bufs=3) as ld,
            tc.tile_pool(name="sq", bufs=3) as sqp,
            tc.tile_pool(name="stat", bufs=4) as stat,
            tc.tile_pool(name="xt", bufs=1) as xt,
            tc.tile_pool(name="bc", bufs=2) as bcp,
            tc.tile_pool(name="outp", bufs=3) as outp,
            tc.tile_pool(name="ps", bufs=6, space="PSUM") as psp,
            tc.tile_pool(name="psb", bufs=2, space="PSUM") as psb,
        ):
            ident_b = const.tile([P, P], BF16)
            make_identity(nc, ident_b)
            ident_f = const.tile([P, P], F32)
            make_identity(nc, ident_f)
            ones128 = const.tile([P, P], F32)
            nc.gpsimd.memset(ones128[:], 1.0)
            ident4 = const.tile([P, 4, P], F32)
            nc.gpsimd.memset(ident4[:], 0.0)
            for b in range(4):
                make_identity(nc, ident4[:, b], nomemset=True)

            x1r = x1.ap().rearrange("(g j p) e -> g p j e", j=4, p=P)
            x2r = x2n.ap().rearrange("(g j p) e -> g p j e", j=4, p=P)

            x1T = [xt.tile([P, n1_local], BF16, tag=f"x1T_{k}", name=f"x1T_{k}")
                   for k in range(kc)]
            x2T = [
                [xt.tile([P, cg_width], BF16, tag=f"x2T_{k}_{cg}",
                         name=f"x2T_{k}_{cg}")
                 for cg in range(n_cgs)]
                for k in range(kc)
            ]

            def row_stats(src_r, g, inv_dst, scale_const):
                """inv_dst [P, 4] = scale / max(row_norm, EPS) for 4 row-tiles."""
                ld_t = ld.tile([P, 4, d], BF16, tag="ld")
                nc.sync.dma_start(ld_t[:], src_r[g])
                ssq = stat.tile([P, 4], F32, tag="ssq")
                for j in range(4):
                    sq_t = sqp.tile([P, d], BF16, tag="sq")
                    nc.scalar.activation(
                        sq_t[:], ld_t[:, j],
                        mybir.ActivationFunctionType.Square,
                        accum_out=ssq[:, j : j + 1],
                    )
                nrm = stat.tile([P, 4], F32, tag="nrm")
                nc.scalar.activation(
                    nrm[:], ssq[:], mybir.ActivationFunctionType.Sqrt
                )
                nc.vector.tensor_scalar_max(nrm[:], nrm[:], EPS)
                nc.vector.reciprocal(inv_dst, nrm[:])
                if scale_const != 1.0:
                    nc.vector.tensor_scalar_mul(inv_dst, inv_dst, scale_const)
                return ld_t

            # ---- x2 per column group: stats -> bcast -> scale ----------
            def prep_cg(cg):
                for k in range(kc):
                    nc.sync.dma_start(
                        x2T[k][cg][:],
                        x2t[k * P : (k + 1) * P,
                            cg * cg_width : (cg + 1) * cg_width],
                    )
                # compact inverse norms for the cg's rows: [P, cg_rt]
                inv2 = stat.tile([P, cg_rt], F32, tag="inv2",
                                 name=f"inv2_{cg}")
                for g2 in range(cg_rt // 4):
                    row_stats(x2r, cg * (cg_rt // 4) + g2,
                              inv2[:, g2 * 4 : (g2 + 1) * 4], 1.0)
                # partition-broadcast: bc[:, c*P+p] = inv2[p, c] via
                # ones128.T @ diag(inv2[:, c]) (column sums of a diagonal)
                bc = bcp.tile([P, cg_width], F32, tag="bc", name=f"bc_{cg}")
                for c0 in range(0, cg_rt, 4):
                    dg4 = stat.tile([P, 4, P], F32, tag="dg4",
                                    name=f"dg4_{cg}_{c0}")
                    nc.vector.tensor_mul(
                        dg4[:], ident4[:],
                        inv2[:, c0 : c0 + 4, None].to_broadcast((P, 4, P)),
                    )
                    ps_b = psb.tile([P, 512], F32, tag="psb",
                                    name=f"psb_{cg}_{c0}")
                    nc.tensor.matmul(ps_b[:], lhsT=ones128[:], rhs=dg4[:],
                                     start=True, stop=True)
                    nc.vector.tensor_copy(
                        bc[:, c0 * P : (c0 + 4) * P], ps_b[:]
                    )
                # scale the transposed operand in place (bf16 * f32 -> bf16)
                for k in range(kc):
                    nc.vector.tensor_mul(
                        x2T[k][cg][:], x2T[k][cg][:], bc[:]
                    )

            def gemm_cg(cg):
                for m in range(m_tiles):
                    pss = [psp.tile([P, 512], F32, tag="ps",
                                    name=f"ps_{cg}_{m}_{j}")
                           for j in range(nch)]
                    for k in range(kc):
                        for j in range(nch):
                            nc.tensor.matmul(
                                pss[j][:],
                                lhsT=x1T[k][:, m * P : (m + 1) * P],
                                rhs=x2T[k][cg][:, j * 512 : (j + 1) * 512],
                                start=(k == 0), stop=(k == kc - 1),
                            )
                    ot = outp.tile([P, cg_width], F32, tag="ot",
                                   name=f"ot_{cg}_{m}")
                    for j in range(nch):
                        dst = ot[:, j * 512 : (j + 1) * 512]
                        if j % 2 == 0:
                            nc.vector.tensor_copy(dst, pss[j][:])
                        else:
                            nc.scalar.copy(dst, pss[j][:])
                    nc.sync.dma_start(
                        out[m * P : (m + 1) * P,
                            cg * cg_width : (cg + 1) * cg_width],
                        ot[:],
                    )

            # ---- x1 (emitted after cg0 prep so ACT/DVE/DMA warm up): stats -> normalize (bf16) -> PE transpose ----------
            for g in range(n1_local // 512):
                inv1 = stat.tile([P, 4], F32, tag="inv1")
                ld_t = row_stats(x1r, g, inv1[:], SCALE)
                x1nrm = sqp.tile([P, 4, d], BF16, tag="x1nrm")
                for j in range(4):
                    nc.vector.tensor_scalar_mul(
                        x1nrm[:, j], ld_t[:, j], inv1[:, j : j + 1]
                    )
                for k in range(kc):
                    ps_t = psb.tile([P, 512], F32, tag="psb")
                    for j in range(4):
                        nc.tensor.matmul(
                            ps_t[:, j * P : (j + 1) * P],
                            lhsT=x1nrm[:, j, k * P : (k + 1) * P],
                            rhs=ident_b[:],
                            start=True, stop=True,
                        )
                    dst = x1T[k][:, g * 512 : (g + 1) * 512]
                    if k % 2 == 0:
                        nc.vector.tensor_copy(dst, ps_t[:])
                    else:
                        nc.scalar.copy(dst, ps_t[:])

            for cg in range(n_cgs):
                prep_cg(cg)
                gemm_cg(cg)

    nc.compile()
    return nc


def _get_program():
    key = "default"
    if key not in _PROGRAM_CACHE:
        _PROGRAM_CACHE[key] = build_program()
    return _PROGRAM_CACHE[key]


def make_in_maps(x1: np.ndarray, x2: np.ndarray) -> list:
    x1 = np.asarray(x1, dtype=np.float32)
    x2 = np.asarray(x2, dtype=np.float32)
    assert x1.shape == (N1, D) and x2.shape == (N2, D), (x1.shape, x2.shape)
    x1_b = x1.astype(ml_dtypes.bfloat16)
    x2_b = x2.astype(ml_dtypes.bfloat16)
    x2t_b = np.ascontiguousarray(x2_b.T)
    rows = N1 // N_CORES
    return [
        {
            "x1": np.ascontiguousarray(x1_b[c * rows : (c + 1) * rows]),
            "x2n": x2_b,
            "x2t": x2t_b,
        }
        for c in range(N_CORES)
    ]


def kernel(x1: np.ndarray, x2: np.ndarray) -> np.ndarray:
    nc = _get_program()
    in_maps = make_in_maps(x1, x2)
    res = run_bass_kernel_spmd(nc, in_maps, core_ids=list(range(N_CORES)))
    return np.concatenate([res.results[c]["out"] for c in range(N_CORES)], axis=0)


if __name__ == "__main__":
    rng = np.random.default_rng(0)
    a = rng.standard_normal((N1, D), dtype=np.float32)
    b = rng.standard_normal((N2, D), dtype=np.float32)
    got = kernel(a, b)
    n1 = np.maximum(np.linalg.norm(a, axis=-1, keepdims=True), EPS)
    n2 = np.maximum(np.linalg.norm(b, axis=-1, keepdims=True), EPS)
    want = SCALE * (a / n1) @ (b / n2).T
    err = np.abs(got - want)
    rel = np.linalg.norm(got - want) / np.linalg.norm(want)
    print(f"max abs err: {err.max():.3e}  rel: {rel:.3e}")



# revision 4
# speedup vs baseline: 1.1461x; 1.1461x over previous
"""Pairwise cosine-similarity kernel for Trainium2 (8 NeuronCores, SPMD).

Computes out = 16 * normalize(x1) @ normalize(x2).T for x1, x2 [8192, 512] f32.

Sharding: x1 rows are split across the 8 cores (1024 rows each); x2 is
replicated. Each core computes its [1024, 8192] slice of the output.

Host-side prep is layout/dtype only: bf16 casts, pre-transposed copies of x1
and x2 (no on-device PE transposes), and an fp8-e4m3 copy of x2 used only as
the norm source. The output travels as fp16 and is upcast to f32 on host.
All FLOPs (norms, normalization, GEMM, scaling) run on device.

Schedule (per core):
  - x1 norms: DVE square-accumulate (scalar_tensor_tensor + accum_out) on the
    natural bf16 copy -> ACT sqrt -> DVE clamp/recip -> inv1 = 16/n1 [128, 8].
  - Per column group (cg = 1024 x2 rows): ACT Square on the fp8 copy -> DVE
    row-sum accums -> sqrt/clamp/recip -> inv2 [128, 8] -> partition
    broadcast via ones-matmul against diag(inv2) -> bc [128, 1024] bf16.
    Stats run TWO cgs ahead so chain latency never gates the PE.
  - GEMM per (cg, m-tile): two psum [128, 512] over 4 K-chunks (bf16).
    Epilogue folds both normalizations into the PSUM->SBUF copy:
      j0: DVE scalar_tensor_tensor   out = (psum * inv1) * bc
      j1: ACT Copy(scale=inv1), with x2T's j1 columns pre-scaled by bc on
          GpSimd one cg ahead.
    Outputs accumulate in [128, 2, 1024] fp16 m-pair tiles, DMA'd from the
    gpsimd queue.
  - cg0 runs k-major in 3-m-tile groups (6 psum banks) so the PE starts on
    the first K-chunks as they land; cg0/cg1 use row-scale-only copies plus a
    second-pass column scale once bc is ready (their bc would otherwise gate
    the pipeline at startup).
"""

import sys

for _p in ("/root/.axon_site/_ro/trn_rl_repo", "/opt/trn_rl_repo"):
    if _p not in sys.path:
        sys.path.append(_p)

import ml_dtypes
import numpy as np

import concourse.bass as bass
import concourse.tile as tile
from concourse import bacc, mybir
from concourse.bass_utils import run_bass_kernel_spmd
from concourse.masks import make_identity

F32 = mybir.dt.float32
BF16 = mybir.dt.bfloat16
FP16 = mybir.dt.float16
FP8 = mybir.dt.float8e4
P = 128
SCALE = 16.0
EPS = 1e-8

N_CORES = 8
N1 = 8192  # x1 rows (total)
N2 = 8192  # x2 rows
D = 512  # feature dim
KC = D // P  # 4 K-chunks
CGW = 1024  # column-group width
N_CGS = N2 // CGW  # 8
MT_N = (N1 // N_CORES) // P  # 8 m-tiles per core

MUL = mybir.AluOpType.mult
ACTF = mybir.ActivationFunctionType

_PROGRAM_CACHE = {}


def build_program():
    n1l = N1 // N_CORES  # 1024 local x1 rows

    nc = bacc.Bacc("TRN2", target_bir_lowering=False, debug=False,
                   num_devices=N_CORES)
    x1n = nc.dram_tensor("x1n", [n1l, D], BF16, kind="ExternalInput")
    x1t = nc.dram_tensor("x1t", [D, n1l], BF16, kind="ExternalInput")
    x2t = nc.dram_tensor("x2t", [D, N2], BF16, kind="ExternalInput")
    x2n8 = nc.dram_tensor("x2n8", [N2, D], FP8, kind="ExternalInput")
    out = nc.dram_tensor("out", [n1l, N2], FP16, kind="ExternalOutput")

    with tile.TileContext(nc) as tc:
        with (
            tc.tile_pool(name="const", bufs=1) as const,
            tc.tile_pool(name="xt", bufs=1) as xt,
            tc.tile_pool(name="sq", bufs=3) as sqp,
            tc.tile_pool(name="stat", bufs=4) as stat,
            tc.tile_pool(name="bc", bufs=3) as bcp,
            tc.tile_pool(name="outp", bufs=6) as outp,
            tc.tile_pool(name="ps", bufs=6, space="PSUM") as psp,
            tc.tile_pool(name="psb", bufs=2, space="PSUM") as psb,
        ):
            ident4 = const.tile([P, 4, P], BF16)
            nc.gpsimd.memset(ident4[:], 0.0)
            for b in range(4):
                make_identity(nc, ident4[:, b], nomemset=True)
            ones128 = const.tile([P, P], BF16)
            nc.gpsimd.memset(ones128[:], 1.0)

            # DRAM access patterns
            x1n_r = x1n.ap().rearrange("(h mt p) e -> h p mt e",
                                       h=2, mt=MT_N // 2, p=P)
            x1t_r = x1t.ap().rearrange("(k p) n -> p k n", k=KC, p=P)
            x2t_r = x2t.ap().rearrange(
                "(k p) (cg n) -> cg p k n", k=KC, p=P, cg=N_CGS, n=CGW
            )
            x2n_r = x2n8.ap().rearrange(
                "(cg mt p) e -> cg p mt e", cg=N_CGS, mt=CGW // P, p=P
            )
            out_r = out.ap().rearrange(
                "(mp mi p) (cg n) -> cg mp p mi n", mi=2, p=P,
                cg=N_CGS, n=CGW
            )

            # Persistent SBUF tiles
            x1T = xt.tile([P, KC, n1l], BF16, name="x1T")
            x2T = [xt.tile([P, KC, CGW], BF16, tag=f"x2T_{cg}",
                           name=f"x2T_{cg}") for cg in range(N_CGS)]
            x1ld = xt.tile([P, MT_N, D], BF16, name="x1ld")
            x2ld = [xt.tile([P, CGW // P, D], FP8, tag=f"x2ld_{cg}",
                            name=f"x2ld_{cg}") for cg in range(N_CGS)]
            inv1 = xt.tile([P, MT_N], F32, name="inv1")
            bc_t = [bcp.tile([P, CGW], BF16, tag="bc", name=f"bc_{cg}")
                    for cg in range(N_CGS)]

            # ---- input DMAs up front (SP queue), priority order ----------
            nc.sync.dma_start(x1ld[:, 0 : MT_N // 2], x1n_r[0])
            nc.sync.dma_start(x1ld[:, MT_N // 2 : MT_N], x1n_r[1])
            nc.sync.dma_start(x1T[:, 0], x1t_r[:, 0])
            nc.sync.dma_start(x2T[0][:, 0], x2t_r[0][:, 0])
            nc.sync.dma_start(x2ld[0][:], x2n_r[0])
            for k in range(1, KC):
                nc.sync.dma_start(x1T[:, k], x1t_r[:, k])
                nc.sync.dma_start(x2T[0][:, k], x2t_r[0][:, k])
            for cg in range(1, N_CGS):
                nc.sync.dma_start(x2ld[cg][:], x2n_r[cg])
                nc.sync.dma_start(x2T[cg][:], x2t_r[cg])

            def finish_stats(ssq, inv_dst, scale_const, tagp):
                """inv_dst = scale / max(sqrt(ssq), EPS)."""
                nrm = stat.tile([P, ssq.shape[1]], F32, tag=f"{tagp}_nrm")
                nc.scalar.activation(nrm[:], ssq[:], ACTF.Sqrt)
                nc.vector.tensor_scalar_max(nrm[:], nrm[:], EPS)
                nc.vector.reciprocal(inv_dst, nrm[:])
                if scale_const != 1.0:
                    nc.vector.tensor_scalar_mul(inv_dst, inv_dst, scale_const)

            # ---- x1 stats: DVE square-accumulate (early, off ACT) --------
            ssq1 = stat.tile([P, MT_N], F32, tag="x1_ssq")
            for mt in range(MT_N):
                dum = sqp.tile([P, D], BF16, tag="x1dum")
                nc.vector.scalar_tensor_tensor(
                    dum[:], x1ld[:, mt], 1.0, x1ld[:, mt],
                    op0=MUL, op1=MUL,
                    accum_out=ssq1[:, mt : mt + 1],
                )
            finish_stats(ssq1, inv1[:], SCALE, "x1")

            def prep_stats(cg):
                """x2 norms for the cg: ACT squares + DVE accums -> inv2."""
                inv2 = stat.tile([P, CGW // P], F32, tag="inv2",
                                 name=f"inv2_{cg}")
                ssq = stat.tile([P, CGW // P], F32, tag="x2_ssq",
                                name=f"x2ssq_{cg}")
                for h in range(2):
                    sq_t = sqp.tile([P, 4, D], BF16, tag="x2sq")
                    nc.scalar.activation(
                        sq_t[:], x2ld[cg][:, h * 4 : (h + 1) * 4],
                        ACTF.Square,
                    )
                    for j in range(4):
                        nc.vector.tensor_scalar(
                            sq_t[:, j], sq_t[:, j], 1.0, 0.0, MUL,
                            mybir.AluOpType.add,
                            accum_out=ssq[:, h * 4 + j : h * 4 + j + 1],
                        )
                finish_stats(ssq, inv2[:], 1.0, "x2")
                return inv2

            def prep_bcast(cg, inv2, prescale):
                """bc_t[cg][p, c*P+q] = inv2[q, c] via ones.T @ diag."""
                bc = bc_t[cg]
                for c0 in range(0, CGW // P, 4):
                    dg4 = stat.tile([P, 4, P], BF16, tag="dg4",
                                    name=f"dg4_{cg}_{c0}")
                    nc.vector.tensor_mul(
                        dg4[:], ident4[:],
                        inv2[:, c0 : c0 + 4, None].to_broadcast((P, 4, P)),
                    )
                    ps_b = psb.tile([P, 512], F32, tag="psb",
                                    name=f"psb_{cg}_{c0}")
                    nc.tensor.matmul(ps_b[:], lhsT=ones128[:], rhs=dg4[:],
                                     start=True, stop=True)
                    nc.scalar.copy(bc[:, c0 * P : (c0 + 4) * P], ps_b[:])
                if prescale:
                    # scale the j1 half of the transposed operand (GpSimd)
                    for k in range(KC):
                        nc.gpsimd.tensor_mul(
                            x2T[cg][:, k, 512:CGW], x2T[cg][:, k, 512:CGW],
                            bc[:, 512:CGW],
                        )

            # out m-pair tiles, keyed (cg, mp)
            def out_tile(cg, mp):
                return outp.tile([P, 2, CGW], FP16, tag="ot",
                                 name=f"ot_{cg}_{mp}")

            def epilogue(cg, mt, ps0, ps1, ot, fold_bc):
                sl = ot[:, mt % 2]
                if fold_bc:
                    nc.vector.scalar_tensor_tensor(
                        sl[:, 0:512], ps0[:], inv1[:, mt : mt + 1],
                        bc_t[cg][:, 0:512], op0=MUL, op1=MUL,
                    )
                else:
                    nc.vector.tensor_scalar_mul(
                        sl[:, 0:512], ps0[:], inv1[:, mt : mt + 1]
                    )
                nc.scalar.activation(
                    sl[:, 512:CGW], ps1[:], ACTF.Copy,
                    scale=inv1[:, mt : mt + 1],
                )

            def gemm_m(cg, mt, ot, mode):
                """m-major GEMM + epilogue for one (cg, m-tile)."""
                ps0 = psp.tile([P, 512], F32, tag="ps", name=f"ps0_{cg}_{mt}")
                ps1 = psp.tile([P, 512], F32, tag="ps", name=f"ps1_{cg}_{mt}")
                for k in range(KC):
                    nc.tensor.matmul(
                        ps0[:], lhsT=x1T[:, k, mt * P : (mt + 1) * P],
                        rhs=x2T[cg][:, k, 0:512],
                        start=(k == 0), stop=(k == KC - 1),
                    )
                    nc.tensor.matmul(
                        ps1[:], lhsT=x1T[:, k, mt * P : (mt + 1) * P],
                        rhs=x2T[cg][:, k, 512:CGW],
                        start=(k == 0), stop=(k == KC - 1),
                    )
                epilogue(cg, mt, ps0, ps1, ot, fold_bc=(mode == "pre"))
                if mode == "pre" and mt % 2 == 1:
                    nc.gpsimd.dma_start(out_r[cg, mt // 2], ot[:])

            def gemm_cg0_group(mts, ots):
                """cg0 startup: k-major over <=3 m-tiles (6 psum banks)."""
                pss = {}
                for mt in mts:
                    pss[mt] = (
                        psp.tile([P, 512], F32, tag="ps", name=f"c0ps0_{mt}"),
                        psp.tile([P, 512], F32, tag="ps", name=f"c0ps1_{mt}"),
                    )
                for k in range(KC):
                    for mt in mts:
                        nc.tensor.matmul(
                            pss[mt][0][:],
                            lhsT=x1T[:, k, mt * P : (mt + 1) * P],
                            rhs=x2T[0][:, k, 0:512],
                            start=(k == 0), stop=(k == KC - 1),
                        )
                        nc.tensor.matmul(
                            pss[mt][1][:],
                            lhsT=x1T[:, k, mt * P : (mt + 1) * P],
                            rhs=x2T[0][:, k, 512:CGW],
                            start=(k == 0), stop=(k == KC - 1),
                        )
                for mt in mts:
                    epilogue(0, mt, pss[mt][0], pss[mt][1], ots[mt // 2],
                             fold_bc=False)

            def second_pass(cg, ots):
                """post-mode: column-scale whole m-pairs by bc, then DMA."""
                for mp, ot in ots.items():
                    nc.vector.tensor_mul(
                        ot[:], ot[:],
                        bc_t[cg][:, None, :].to_broadcast((P, 2, CGW)),
                    )
                    nc.gpsimd.dma_start(out_r[cg, mp], ot[:])

            # ---- emission schedule --------------------------------------
            ots0 = {mp: out_tile(0, mp) for mp in range(4)}
            gemm_cg0_group([0, 1, 2], ots0)
            inv2_0 = prep_stats(0)
            gemm_cg0_group([3, 4, 5], ots0)
            inv2_1 = prep_stats(1)
            gemm_cg0_group([6, 7], ots0)
            prep_bcast(0, inv2_0, prescale=False)
            second_pass(0, ots0)
            prep_bcast(1, inv2_1, prescale=False)
            inv2_next = prep_stats(2)

            for cg in range(1, N_CGS):
                mode = "post" if cg == 1 else "pre"
                ots = {mp: out_tile(cg, mp) for mp in range(4)}
                for mt in range(0, 4):
                    gemm_m(cg, mt, ots[mt // 2], mode)
                if cg + 1 < N_CGS:
                    prep_bcast(cg + 1, inv2_next,
                               prescale=(cg + 1 >= 2))
                if cg + 2 < N_CGS:
                    inv2_next = prep_stats(cg + 2)
                for mt in range(4, MT_N):
                    gemm_m(cg, mt, ots[mt // 2], mode)
                if mode == "post":
                    second_pass(cg, ots)

    nc.compile()
    return nc


def _get_program():
    key = "default"
    if key not in _PROGRAM_CACHE:
        _PROGRAM_CACHE[key] = build_program()
    return _PROGRAM_CACHE[key]


def make_in_maps(x1: np.ndarray, x2: np.ndarray) -> list:
    x1 = np.asarray(x1, dtype=np.float32)
    x2 = np.asarray(x2, dtype=np.float32)
    assert x1.shape == (N1, D) and x2.shape == (N2, D), (x1.shape, x2.shape)
    fp8_np = mybir.dt.np(FP8)
    x1_b = x1.astype(ml_dtypes.bfloat16)
    x2_b = x2.astype(ml_dtypes.bfloat16)
    x2t_b = np.ascontiguousarray(x2_b.T)
    x2n8 = x2.astype(fp8_np)
    rows = N1 // N_CORES
    maps = []
    for c in range(N_CORES):
        sl = x1_b[c * rows : (c + 1) * rows]
        maps.append({
            "x1n": np.ascontiguousarray(sl),
            "x1t": np.ascontiguousarray(sl.T),
            "x2t": x2t_b,
            "x2n8": x2n8,
        })
    return maps


def kernel(x1: np.ndarray, x2: np.ndarray) -> np.ndarray:
    nc = _get_program()
    in_maps = make_in_maps(x1, x2)
    res = run_bass_kernel_spmd(nc, in_maps, core_ids=list(range(N_CORES)))
    return np.concatenate(
        [res.results[c]["out"] for c in range(N_CORES)], axis=0
    ).astype(np.float32)


if __name__ == "__main__":
    rng = np.random.default_rng(0)
    a = rng.standard_normal((N1, D), dtype=np.float32)
    b = rng.standard_normal((N2, D), dtype=np.float32)
    got = kernel(a, b)
    n1 = np.maximum(np.linalg.norm(a, axis=-1, keepdims=True), EPS)
    n2 = np.maximum(np.linalg.norm(b, axis=-1, keepdims=True), EPS)
    want = SCALE * (a / n1) @ (b / n2).T
    err = np.abs(got - want)
    rel = np.linalg.norm(got - want) / np.linalg.norm(want)
    print(f"max abs err: {err.max():.3e}  rel: {rel:.3e}")
